# revision 17
# baseline (speedup 1.0000x reference)
"""Multi-head attention (B=4, L=1024, D=1024, H=16) on 8 TRN2 NeuronCores.

Sharding: pure data-parallel over (batch, query-half) — core c handles batch
c//2, query rows [512*(c%2), 512*(c%2+1)). Each core computes Q/K/V
projections for its batch (K/V duplicated across the 2 cores of a batch),
full attention for its 512 queries, and the output projection for its slice.
No collectives; the host concatenates the 8 output slices.

Everything on-device is kept in transposed layouts so no transposes are ever
needed:
  Q^T[vd, q]  = Wq(lhsT) @ qT(rhs)         (+bq per-partition via activation)
  K^T[vd, k]  = Wk(lhsT) @ xT(rhs)         (+bk per-partition)
  V  [k, vd]  = xT(lhsT) @ Wv(rhs)         (+bv via K=1 ones-row matmul)
  S^T[k, q]   = K^T_h(lhsT, K=64) @ Q^T_h  per head
  expS        = exp(S^T/8 + kmask_bias)    (ScalarE, PSUM->SBUF bf16)
  O^T+denom   = V_aug(lhsT, M=65) @ expS   (V cols + ones col per head)
  scale       = q_mask * 1/denom, broadcast 16->64 partitions via selector MM
  out[q, d]   = O^T_scaled(lhsT) @ Wo      (+bo via K=1 ones-row matmul)
"""

import os

os.environ.setdefault("MYCRO_LOCAL_CACHE", "1")

import numpy as np
import ml_dtypes

BF16 = ml_dtypes.bfloat16

B, LQ, LK = 4, 1024, 1024
D = 1024  # QD = KD = VD
H, DH = 16, 64
QS = 512  # queries per core
NCORES = 8
NEG = -1e4  # additive key-mask bias (exp(-1e4) == 0 in f32)

_NC_CACHE = {}


def _build_nc(debug_taps=False):
    import concourse.bacc as bacc
    import concourse.mybir as mybir
    import concourse.tile as tile

    dt = mybir.dt
    AF = mybir.ActivationFunctionType

    nc = bacc.Bacc(
        "TRN2",
        debug=False,
        target_bir_lowering=False,
        num_devices=NCORES,
    )

    def din(name, shape, dtype):
        return nc.dram_tensor(name, shape, dtype, kind="ExternalInput").ap()

    qT_d = din("qT", [D, QS], dt.bfloat16)
    xT_d = din("xT", [D, LK], dt.bfloat16)
    wq_d = din("Wq", [D, D], dt.bfloat16)
    wk_d = din("Wk", [D, D], dt.bfloat16)
    wv_d = din("Wv", [D, D], dt.bfloat16)
    wo_d = din("Wo", [D, D], dt.bfloat16)
    bq_d = din("bqp", [128, 8], dt.float32)  # bq[t*128+p] at [p, t]
    bk_d = din("bkp", [128, 8], dt.float32)
    bv_d = din("bvr", [1, D], dt.bfloat16)
    bo_d = din("bor", [1, D], dt.bfloat16)
    kb_d = din("kb", [128, 8], dt.float32)  # key-mask bias, [p, t]
    qm_d = din("qm2", [1, 2 * QS], dt.float32)  # q_mask, doubled row
    out_d = nc.dram_tensor("out", [QS, D], dt.float32, kind="ExternalOutput").ap()

    taps = None
    if debug_taps:
        taps = {
            "dbg_qTp": nc.dram_tensor("dbg_qTp", [128, QS], dt.bfloat16, kind="ExternalOutput").ap(),
            "dbg_kT": nc.dram_tensor("dbg_kT", [128, LK], dt.bfloat16, kind="ExternalOutput").ap(),
            "dbg_v": nc.dram_tensor("dbg_v", [128, H * (DH + 1)], dt.bfloat16, kind="ExternalOutput").ap(),
            "dbg_es": nc.dram_tensor("dbg_es", [128, QS], dt.bfloat16, kind="ExternalOutput").ap(),
            "dbg_oTu": nc.dram_tensor("dbg_oTu", [128, QS], dt.bfloat16, kind="ExternalOutput").ap(),
            "dbg_oTs": nc.dram_tensor("dbg_oTs", [128, QS], dt.bfloat16, kind="ExternalOutput").ap(),
        }

    with tile.TileContext(nc) as tc:
        _body(tc, dt, AF, qT_d, xT_d, wq_d, wk_d, wv_d, wo_d, bq_d, bk_d,
              bv_d, bo_d, kb_d, qm_d, out_d, taps)

    nc.compile()
    return nc


def _body(tc, dt, AF, qT_d, xT_d, wq_d, wk_d, wv_d, wo_d, bq_d, bk_d,
          bv_d, bo_d, kb_d, qm_d, out_d, taps=None):
    from contextlib import ExitStack

    nc = tc.nc
    with ExitStack() as ctx:
        const = ctx.enter_context(tc.tile_pool(name="const", bufs=1))
        espool = ctx.enter_context(tc.tile_pool(name="es", bufs=8))
        psum = ctx.enter_context(tc.tile_pool(name="psum", bufs=8, space="PSUM"))
        opool = ctx.enter_context(tc.tile_pool(name="osb", bufs=3))

        def ctile(shape, dtype, tag):
            return const.tile(shape, dtype, tag=tag, name=tag)

        # ---- persistent SBUF tensors + input DMA ----
        qT_sb = [ctile([128, QS], dt.bfloat16, f"qT{t}") for t in range(8)]
        wq_sb = [ctile([128, D], dt.bfloat16, f"wq{t}") for t in range(8)]
        xT_sb = [ctile([128, LK], dt.bfloat16, f"xT{t}") for t in range(8)]
        wk_sb = [ctile([128, D], dt.bfloat16, f"wk{t}") for t in range(8)]
        wv_sb = [ctile([128, D], dt.bfloat16, f"wv{t}") for t in range(8)]
        wo_sb = [ctile([128, D], dt.bfloat16, f"wo{t}") for t in range(8)]
        for t in range(8):
            r = slice(128 * t, 128 * (t + 1))
            nc.sync.dma_start(qT_sb[t][:], qT_d[r, :])
            nc.sync.dma_start(wq_sb[t][:], wq_d[r, :])
        for t in range(8):
            r = slice(128 * t, 128 * (t + 1))
            nc.sync.dma_start(xT_sb[t][:], xT_d[r, :])
            nc.sync.dma_start(wk_sb[t][:], wk_d[r, :])
        for t in range(8):
            r = slice(128 * t, 128 * (t + 1))
            nc.sync.dma_start(wv_sb[t][:], wv_d[r, :])

        bq_sb = ctile([128, 8], dt.float32, "bq")
        bk_sb = ctile([128, 8], dt.float32, "bk")
        bv_sb = ctile([1, D], dt.bfloat16, "bv")
        bo_sb = ctile([1, D], dt.bfloat16, "bo")
        kb_sb = ctile([128, 8], dt.float32, "kb")
        qm_sb = ctile([1, 2 * QS], dt.float32, "qm")
        nc.sync.dma_start(bq_sb[:], bq_d[:, :])
        nc.sync.dma_start(bk_sb[:], bk_d[:, :])
        nc.sync.dma_start(bv_sb[:], bv_d[:, :])
        nc.sync.dma_start(bo_sb[:], bo_d[:, :])
        nc.sync.dma_start(kb_sb[:], kb_d[:, :])
        nc.sync.dma_start(qm_sb[:], qm_d[:, :])
        for t in range(8):
            r = slice(128 * t, 128 * (t + 1))
            nc.sync.dma_start(wo_sb[t][:], wo_d[r, :])

        ones1 = ctile([1, 128], dt.bfloat16, "ones1")
        nc.gpsimd.memset(ones1[:], 1.0)

        # ---- Q^T projection: qTp[j] = (Wq.T @ q.T)[128j:128j+128, :] + bq ----
        qTp = [ctile([128, QS], dt.bfloat16, f"qTp{j}") for j in range(8)]
        for j in range(8):
            ps = psum.tile([128, QS], dt.float32, tag="ps", name="ps")
            for kt in range(8):
                nc.tensor.matmul(
                    ps[:], wq_sb[kt][:, 128 * j:128 * (j + 1)], qT_sb[kt][:],
                    start=(kt == 0), stop=(kt == 7))
            nc.scalar.activation(qTp[j][:], ps[:], AF.Identity,
                                 bias=bq_sb[:, j:j + 1], scale=1.0)

        # ---- K^T projection: kT[j][:, :] = (Wk.T @ x.T)[128j:...] + bk ----
        kT_sb = [ctile([128, LK], dt.bfloat16, f"kT{j}") for j in range(8)]
        for j in range(8):
            for n in range(2):
                c = slice(512 * n, 512 * (n + 1))
                ps = psum.tile([128, 512], dt.float32, tag="ps", name="ps")
                for kt in range(8):
                    nc.tensor.matmul(
                        ps[:], wk_sb[kt][:, 128 * j:128 * (j + 1)], xT_sb[kt][:, c],
                        start=(kt == 0), stop=(kt == 7))
                nc.scalar.activation(kT_sb[j][:, c], ps[:], AF.Identity,
                                     bias=bk_sb[:, j:j + 1], scale=1.0)

        # ---- V projection into V_aug layout: per k-tile [128, 16*(64+1)] ----
        # head h occupies cols [65h, 65h+64) = V[:, 64h:64h+64]; col 65h+64 = 1.
        v_sb = [ctile([128, H * (DH + 1)], dt.bfloat16, f"v{t}") for t in range(8)]
        for t in range(8):
            ones_cols = v_sb[t][:].rearrange("p (h c) -> p h c", c=DH + 1)[:, :, DH:DH + 1]
            nc.gpsimd.memset(ones_cols, 1.0)
        for t in range(8):
            for n in range(2):
                c = slice(512 * n, 512 * (n + 1))
                ps = psum.tile([128, 512], dt.float32, tag="ps", name="ps")
                for kd in range(8):
                    nc.tensor.matmul(
                        ps[:], xT_sb[kd][:, 128 * t:128 * (t + 1)], wv_sb[kd][:, c],
                        start=(kd == 0), stop=False)
                nc.tensor.matmul(ps[:], ones1[:], bv_sb[:, c],
                                 start=False, stop=True)
                for i in range(8):
                    h = 8 * n + i
                    nc.vector.tensor_copy(
                        v_sb[t][:, 65 * h:65 * h + 64], ps[:, 64 * i:64 * (i + 1)])

        # ---- attention, one head-pair (2j, 2j+1) at a time ----
        # Engine-op APs must start at partition 0/32/64/96, so per-head
        # denominator rows are moved with DMA and the scale math runs on the
        # whole [16, QS] tile after all pairs.
        oTu = [ctile([128, QS], dt.bfloat16, f"oTu{j}") for j in range(8)]
        oTs = [ctile([128, QS], dt.bfloat16, f"oTs{j}") for j in range(8)]
        dscr = ctile([1, 2 * QS], dt.float32, "dscr")  # pair denoms, free axis
        sca = ctile([1, 2 * QS], dt.float32, "sca")  # q_mask / denom
        scb = ctile([1, 2 * QS], dt.bfloat16, "scb")
        ones64 = ctile([1, 64], dt.bfloat16, "ones64")
        nc.gpsimd.memset(ones64[:], 1.0)

        for j in range(8):
            hA, hB = 2 * j, 2 * j + 1
            oA = psum.tile([128, QS], dt.float32, tag="ps", name="ps")
            oB = psum.tile([128, QS], dt.float32, tag="ps", name="ps")
            for kt in range(8):
                kc = slice(128 * kt, 128 * (kt + 1))
                sA = psum.tile([128, QS], dt.float32, tag="ps", name="ps")
                nc.tensor.matmul(sA[:], kT_sb[j][0:64, kc], qTp[j][0:64, :],
                                 start=True, stop=True)
                eA = espool.tile([128, QS], dt.bfloat16, tag="es", name="es")
                nc.scalar.activation(eA[:], sA[:], AF.Exp,
                                     bias=kb_sb[:, kt:kt + 1], scale=0.125)
                if taps is not None and j == 0 and kt == 0:
                    nc.sync.dma_start(taps["dbg_es"][:, :], eA[:])
                nc.tensor.matmul(oA[0:65, :], v_sb[kt][:, 65 * hA:65 * hA + 65],
                                 eA[:], start=(kt == 0), stop=(kt == 7))

                sB = psum.tile([128, QS], dt.float32, tag="ps", name="ps")
                nc.tensor.matmul(sB[:], kT_sb[j][64:128, kc], qTp[j][64:128, :],
                                 start=True, stop=True)
                eB = espool.tile([128, QS], dt.bfloat16, tag="es", name="es")
                nc.scalar.activation(eB[:], sB[:], AF.Exp,
                                     bias=kb_sb[:, kt:kt + 1], scale=0.125)
                nc.tensor.matmul(oB[0:65, :], v_sb[kt][:, 65 * hB:65 * hB + 65],
                                 eB[:], start=(kt == 0), stop=(kt == 7))

            # row 64 = denominator; rows 0-63 = unnormalized O^T
            nc.vector.tensor_copy(dscr[0:1, 0:QS], oA[64:65, :])
            nc.vector.tensor_copy(dscr[0:1, QS:2 * QS], oB[64:65, :])
            nc.vector.tensor_copy(oTu[j][0:64, :], oA[0:64, :])
            nc.vector.tensor_copy(oTu[j][64:128, :], oB[0:64, :])

            # scale rows (partition 0): q_mask / denom, cast to bf16
            nc.vector.reciprocal(sca[:], dscr[:])
            nc.vector.tensor_mul(scb[:], sca[:], qm_sb[:])
            # broadcast each scale row to 64 partitions: ones64 outer product
            sr = psum.tile([128, QS], dt.float32, tag="ps", name="ps")
            nc.tensor.matmul(sr[0:64, :], ones64[:], scb[:, 0:QS],
                             start=True, stop=True)
            nc.tensor.matmul(sr[64:128, :], ones64[:], scb[:, QS:2 * QS],
                             start=True, stop=True, tile_position=(0, 64))
            nc.vector.tensor_mul(oTs[j][:], oTu[j][:], sr[:])

        if taps is not None:
            nc.sync.dma_start(taps["dbg_qTp"][:, :], qTp[0][:])
            nc.sync.dma_start(taps["dbg_kT"][:, :], kT_sb[0][:])
            nc.sync.dma_start(taps["dbg_v"][:, :], v_sb[0][:])
            nc.sync.dma_start(taps["dbg_oTu"][:, :], oTu[0][:])
            nc.sync.dma_start(taps["dbg_oTs"][:, :], oTs[0][:])

        # ---- output projection: out[q, d] = O^T.T @ Wo + bo ----
        for qt in range(4):
            qr = slice(128 * qt, 128 * (qt + 1))
            for n in range(2):
                c = slice(512 * n, 512 * (n + 1))
                ps = psum.tile([128, 512], dt.float32, tag="ps", name="ps")
                for j in range(8):
                    nc.tensor.matmul(ps[:], oTs[j][:, qr], wo_sb[j][:, c],
                                     start=(j == 0), stop=False)
                nc.tensor.matmul(ps[:], ones1[:], bo_sb[:, c],
                                 start=False, stop=True)
                ot = opool.tile([128, 512], dt.float32, tag="osb", name="osb")
                nc.scalar.copy(ot[:], ps[:])
                nc.sync.dma_start(out_d[qr, c], ot[:])


def get_nc():
    if "nc" not in _NC_CACHE:
        _NC_CACHE["nc"] = _build_nc()
    return _NC_CACHE["nc"]


def make_in_maps(q, x, q_mask, k_mask, Wq, bq, Wk, bk, Wv, bv, Wo, bo):
    """Host-side shard/layout prep. Returns in_maps for cores 0..7."""
    wq_b = Wq.astype(BF16)
    wk_b = Wk.astype(BF16)
    wv_b = Wv.astype(BF16)
    wo_b = Wo.astype(BF16)
    bq_p = np.ascontiguousarray(bq.astype(np.float32).reshape(8, 128).T)
    bk_p = np.ascontiguousarray(bk.astype(np.float32).reshape(8, 128).T)
    bv_r = bv.astype(BF16).reshape(1, D)
    bo_r = bo.astype(BF16).reshape(1, D)
    in_maps = []
    for c in range(NCORES):
        b, qh = c // 2, c % 2
        qs = slice(QS * qh, QS * (qh + 1))
        kbias = np.where(k_mask[b] != 0, 0.0, NEG).astype(np.float32)
        in_maps.append({
            "qT": np.ascontiguousarray(q[b, qs, :].T).astype(BF16),
            "xT": np.ascontiguousarray(x[b].T).astype(BF16),
            "Wq": wq_b, "Wk": wk_b, "Wv": wv_b, "Wo": wo_b,
            "bqp": bq_p, "bkp": bk_p, "bvr": bv_r, "bor": bo_r,
            "kb": np.ascontiguousarray(kbias.reshape(8, 128).T),
            "qm2": np.tile(q_mask[b, qs].astype(np.float32), 2)[None, :],
        })
    return in_maps


def kernel(q, x, q_mask, k_mask, Wq, bq, Wk, bk, Wv, bv, Wo, bo):
    from concourse import bass_utils

    q = np.asarray(q, np.float32)
    x = np.asarray(x, np.float32)
    q_mask = np.asarray(q_mask)
    k_mask = np.asarray(k_mask)

    nc = get_nc()
    in_maps = make_in_maps(q, x, q_mask, k_mask, Wq, bq, Wk, bk, Wv, bv, Wo, bo)
    res = bass_utils.run_bass_kernel_spmd(nc, in_maps, core_ids=list(range(NCORES)))

    out = np.empty((B, LQ, D), np.float32)
    for c in range(NCORES):
        b, qh = c // 2, c % 2
        out[b, QS * qh:QS * (qh + 1), :] = res.results[c]["out"]
    return out


# revision 20
# speedup vs baseline: 1.0461x; 1.0461x over previous
"""Multi-head attention (B=4, L=1024, D=1024, H=16) on 8 TRN2 NeuronCores.

Sharding: pure data-parallel over (batch, query-half) — core c handles batch
c//2, query rows [512*(c%2), 512*(c%2+1)). Each core computes Q/K/V
projections for its batch (K/V duplicated across the 2 cores of a batch),
full attention for its 512 queries, and the output projection for its slice.
No collectives; the host concatenates the 8 output slices.

Everything on-device is kept in transposed layouts so no transposes are ever
needed:
  Q^T[vd, q]  = Wq(lhsT) @ qT(rhs)         (+bq per-partition via activation)
  K^T[vd, k]  = Wk(lhsT) @ xT(rhs)         (+bk per-partition)
  V  [k, vd]  = xT(lhsT) @ Wv(rhs)         (+bv via K=1 ones-row matmul)
  S^T[k, q]   = K^T_h(lhsT, K=64) @ Q^T_h  per head
  expS        = exp(S^T/8 + kmask_bias)    (ScalarE, PSUM->SBUF bf16)
  O^T+denom   = V_aug(lhsT, M=65) @ expS   (V cols + ones col per head)
  scale       = q_mask * 1/denom, broadcast 16->64 partitions via selector MM
  out[q, d]   = O^T_scaled(lhsT) @ Wo      (+bo via K=1 ones-row matmul)
"""

import os

os.environ.setdefault("MYCRO_LOCAL_CACHE", "1")

import numpy as np
import ml_dtypes

BF16 = ml_dtypes.bfloat16

B, LQ, LK = 4, 1024, 1024
D = 1024  # QD = KD = VD
H, DH = 16, 64
QS = 512  # queries per core
NCORES = 8
NEG = -1e4  # additive key-mask bias (exp(-1e4) == 0 in f32)

_NC_CACHE = {}


def _build_nc(debug_taps=False):
    import concourse.bacc as bacc
    import concourse.mybir as mybir
    import concourse.tile as tile

    dt = mybir.dt
    AF = mybir.ActivationFunctionType

    nc = bacc.Bacc(
        "TRN2",
        debug=False,
        target_bir_lowering=False,
        num_devices=NCORES,
    )

    def din(name, shape, dtype):
        return nc.dram_tensor(name, shape, dtype, kind="ExternalInput").ap()

    qT_d = din("qT", [D, QS], dt.bfloat16)
    xT_d = din("xT", [D, LK], dt.bfloat16)
    wq_d = din("Wq", [D, D], dt.bfloat16)
    wk_d = din("Wk", [D, D], dt.bfloat16)
    wv_d = din("Wv", [D, D], dt.bfloat16)
    wo_d = din("Wo", [D, D], dt.bfloat16)
    bq_d = din("bqp", [128, 8], dt.float32)  # bq[t*128+p] at [p, t]
    bk_d = din("bkp", [128, 8], dt.float32)
    bv_d = din("bvr", [1, D], dt.bfloat16)
    bo_d = din("bor", [1, D], dt.bfloat16)
    kb_d = din("kb", [128, 8], dt.float32)  # key-mask bias, [p, t]
    qm_d = din("qmc", [128, 4], dt.float32)  # q_mask, [p, qt]
    out_d = nc.dram_tensor("out", [QS, D], dt.float32, kind="ExternalOutput").ap()

    taps = None
    if debug_taps:
        taps = {
            "dbg_qTp": nc.dram_tensor("dbg_qTp", [128, QS], dt.bfloat16, kind="ExternalOutput").ap(),
            "dbg_kT": nc.dram_tensor("dbg_kT", [128, LK], dt.bfloat16, kind="ExternalOutput").ap(),
            "dbg_v": nc.dram_tensor("dbg_v", [128, H * (DH + 1)], dt.bfloat16, kind="ExternalOutput").ap(),
            "dbg_es": nc.dram_tensor("dbg_es", [128, QS], dt.bfloat16, kind="ExternalOutput").ap(),
            "dbg_oTs": nc.dram_tensor("dbg_oTs", [128, QS], dt.bfloat16, kind="ExternalOutput").ap(),
        }

    with tile.TileContext(nc) as tc:
        _body(tc, dt, AF, qT_d, xT_d, wq_d, wk_d, wv_d, wo_d, bq_d, bk_d,
              bv_d, bo_d, kb_d, qm_d, out_d, taps)

    nc.compile()
    return nc


def _body(tc, dt, AF, qT_d, xT_d, wq_d, wk_d, wv_d, wo_d, bq_d, bk_d,
          bv_d, bo_d, kb_d, qm_d, out_d, taps=None):
    from contextlib import ExitStack

    import concourse.mybir as mybir

    ALU = mybir.AluOpType
    nc = tc.nc
    with ExitStack() as ctx:
        const = ctx.enter_context(tc.tile_pool(name="const", bufs=1))
        espool = ctx.enter_context(tc.tile_pool(name="es", bufs=8))
        psum = ctx.enter_context(tc.tile_pool(name="psum", bufs=8, space="PSUM"))
        opool = ctx.enter_context(tc.tile_pool(name="osb", bufs=3))
        srpool = ctx.enter_context(tc.tile_pool(name="srp", bufs=2))

        def ctile(shape, dtype, tag):
            return const.tile(shape, dtype, tag=tag, name=tag)

        # ---- persistent SBUF tensors + input DMA ----
        # order matters: V/K projections start as soon as xT/wv/wk tiles land
        qT_sb = [ctile([128, QS], dt.bfloat16, f"qT{t}") for t in range(8)]
        wq_sb = [ctile([128, D], dt.bfloat16, f"wq{t}") for t in range(8)]
        xT_sb = [ctile([128, LK], dt.bfloat16, f"xT{t}") for t in range(8)]
        wk_sb = [ctile([128, D], dt.bfloat16, f"wk{t}") for t in range(8)]
        wv_sb = [ctile([128, D], dt.bfloat16, f"wv{t}") for t in range(8)]
        wo_sb = [ctile([128, D], dt.bfloat16, f"wo{t}") for t in range(8)]
        for t in range(8):
            r = slice(128 * t, 128 * (t + 1))
            nc.sync.dma_start(xT_sb[t][:], xT_d[r, :])
            nc.sync.dma_start(wv_sb[t][:], wv_d[r, :])
            nc.sync.dma_start(wk_sb[t][:], wk_d[r, :])
        for t in range(8):
            r = slice(128 * t, 128 * (t + 1))
            nc.sync.dma_start(qT_sb[t][:], qT_d[r, :])
            nc.sync.dma_start(wq_sb[t][:], wq_d[r, :])

        bq_sb = ctile([128, 8], dt.float32, "bq")
        bk_sb = ctile([128, 8], dt.float32, "bk")
        bv_sb = ctile([1, D], dt.bfloat16, "bv")
        bo_sb = ctile([1, D], dt.bfloat16, "bo")
        kb_sb = ctile([128, 8], dt.float32, "kb")
        qm_sb = ctile([128, 4], dt.float32, "qm")  # q_mask, [p, qt]
        nc.sync.dma_start(bq_sb[:], bq_d[:, :])
        nc.sync.dma_start(bk_sb[:], bk_d[:, :])
        nc.sync.dma_start(bv_sb[:], bv_d[:, :])
        nc.sync.dma_start(bo_sb[:], bo_d[:, :])
        nc.sync.dma_start(kb_sb[:], kb_d[:, :])
        nc.sync.dma_start(qm_sb[:], qm_d[:, :])
        for t in range(8):
            r = slice(128 * t, 128 * (t + 1))
            nc.sync.dma_start(wo_sb[t][:], wo_d[r, :])

        ones1 = ctile([1, 128], dt.bfloat16, "ones1")
        nc.gpsimd.memset(ones1[:], 1.0)

        # bo broadcast to all partitions (final tiles add it with DVE)
        bo_rep = ctile([128, D], dt.float32, "bo_rep")
        for n in range(2):
            c = slice(512 * n, 512 * (n + 1))
            ps = psum.tile([128, 512], dt.float32, tag="ps", name="ps")
            nc.tensor.matmul(ps[:], ones1[:], bo_sb[:, c], start=True, stop=True)
            nc.vector.tensor_copy(bo_rep[:, c], ps[:])

        # ---- V projection into V_aug layout: per k-tile [128, 16*(64+1)] ----
        # head h occupies cols [65h, 65h+64) = V[:, 64h:64h+64]; col 65h+64 = 1.
        v_sb = [ctile([128, H * (DH + 1)], dt.bfloat16, f"v{t}") for t in range(8)]
        for t in range(8):
            ones_cols = v_sb[t][:].rearrange("p (h c) -> p h c", c=DH + 1)[:, :, DH:DH + 1]
            nc.gpsimd.memset(ones_cols, 1.0)
        for t in range(8):
            for n in range(2):
                c = slice(512 * n, 512 * (n + 1))
                ps = psum.tile([128, 512], dt.float32, tag="ps", name="ps")
                for kd in range(8):
                    nc.tensor.matmul(
                        ps[:], xT_sb[kd][:, 128 * t:128 * (t + 1)], wv_sb[kd][:, c],
                        start=(kd == 0), stop=False)
                nc.tensor.matmul(ps[:], ones1[:], bv_sb[:, c],
                                 start=False, stop=True)
                for i in range(8):
                    h = 8 * n + i
                    nc.vector.tensor_copy(
                        v_sb[t][:, 65 * h:65 * h + 64], ps[:, 64 * i:64 * (i + 1)])

        # ---- per head-pair: K^T/Q^T projection for its vd-tile, then attention
        kT_sb = [ctile([128, LK], dt.bfloat16, f"kT{j}") for j in range(8)]
        qTp = [ctile([128, QS], dt.bfloat16, f"qTp{j}") for j in range(8)]
        oTs = [ctile([128, QS], dt.bfloat16, f"oTs{j}") for j in range(8)]
        dscr = ctile([1, 2 * QS], dt.float32, "dscr")
        sca = ctile([1, 2 * QS], dt.float32, "sca")
        rscr = ctile([1, 2 * QS], dt.float32, "rscr")
        ones64f = ctile([1, 64], dt.float32, "ones64f")
        nc.gpsimd.memset(ones64f[:], 1.0)

        for j in range(8):
            hA, hB = 2 * j, 2 * j + 1
            # K^T tile j (+bk via DVE per-partition add)
            for n in range(2):
                c = slice(512 * n, 512 * (n + 1))
                ps = psum.tile([128, 512], dt.float32, tag="ps", name="ps")
                for kt in range(8):
                    nc.tensor.matmul(
                        ps[:], wk_sb[kt][:, 128 * j:128 * (j + 1)], xT_sb[kt][:, c],
                        start=(kt == 0), stop=(kt == 7))
                nc.vector.tensor_scalar_add(kT_sb[j][:, c], ps[:], bq_or(bk_sb, j))
            # Q^T tile j (+bq)
            ps = psum.tile([128, QS], dt.float32, tag="ps", name="ps")
            for kt in range(8):
                nc.tensor.matmul(
                    ps[:], wq_sb[kt][:, 128 * j:128 * (j + 1)], qT_sb[kt][:],
                    start=(kt == 0), stop=(kt == 7))
            nc.vector.tensor_scalar_add(qTp[j][:], ps[:], bq_or(bq_sb, j))

            # attention for heads (2j, 2j+1)
            oA = psum.tile([128, QS], dt.float32, tag="ps", name="ps")
            oB = psum.tile([128, QS], dt.float32, tag="ps", name="ps")
            for kt in range(8):
                kc = slice(128 * kt, 128 * (kt + 1))
                sA = psum.tile([128, QS], dt.float32, tag="ps", name="ps")
                nc.tensor.matmul(sA[:], kT_sb[j][0:64, kc], qTp[j][0:64, :],
                                 start=True, stop=True)
                eA = espool.tile([128, QS], dt.bfloat16, tag="es", name="es")
                nc.scalar.activation(eA[:], sA[:], AF.Exp,
                                     bias=kb_sb[:, kt:kt + 1], scale=0.125)
                if taps is not None and j == 0 and kt == 0:
                    nc.sync.dma_start(taps["dbg_es"][:, :], eA[:])
                nc.tensor.matmul(oA[0:65, :], v_sb[kt][:, 65 * hA:65 * hA + 65],
                                 eA[:], start=(kt == 0), stop=(kt == 7))

                sB = psum.tile([128, QS], dt.float32, tag="ps", name="ps")
                nc.tensor.matmul(sB[:], kT_sb[j][64:128, kc], qTp[j][64:128, :],
                                 start=True, stop=True)
                eB = espool.tile([128, QS], dt.bfloat16, tag="es", name="es")
                nc.scalar.activation(eB[:], sB[:], AF.Exp,
                                     bias=kb_sb[:, kt:kt + 1], scale=0.125)
                nc.tensor.matmul(oB[0:65, :], v_sb[kt][:, 65 * hB:65 * hB + 65],
                                 eB[:], start=(kt == 0), stop=(kt == 7))

            # row 64 = denominator; 1/denom via ScalarE LUT, broadcast to the
            # 64 partitions of each head with a K=1 ones matmul, then apply
            nc.vector.tensor_copy(dscr[0:1, 0:QS], oA[64:65, :])
            nc.vector.tensor_copy(dscr[0:1, QS:2 * QS], oB[64:65, :])
            nc.vector.reciprocal_approx_accurate(out=sca[:], in_=dscr[:],
                                                 scratch=rscr[:])
            sr = psum.tile([128, QS], dt.float32, tag="ps", name="ps")
            nc.tensor.matmul(sr[0:64, :], ones64f[:], sca[:, 0:QS],
                             start=True, stop=True)
            nc.tensor.matmul(sr[64:128, :], ones64f[:], sca[:, QS:2 * QS],
                             start=True, stop=True, tile_position=(0, 64))
            srs = srpool.tile([128, QS], dt.float32, tag="srs", name="srs")
            nc.vector.tensor_copy(srs[:], sr[:])
            nc.vector.tensor_mul(oTs[j][0:64, :], oA[0:64, :], srs[0:64, :])
            nc.vector.tensor_mul(oTs[j][64:128, :], oB[0:64, :], srs[64:128, :])

        if taps is not None:
            nc.sync.dma_start(taps["dbg_qTp"][:, :], qTp[0][:])
            nc.sync.dma_start(taps["dbg_kT"][:, :], kT_sb[0][:])
            nc.sync.dma_start(taps["dbg_v"][:, :], v_sb[0][:])
            nc.sync.dma_start(taps["dbg_oTs"][:, :], oTs[0][:])

        # ---- output projection: out[q, d] = (O^T.T @ Wo) * q_mask + bo ----
        for qt in range(4):
            qr = slice(128 * qt, 128 * (qt + 1))
            for n in range(2):
                c = slice(512 * n, 512 * (n + 1))
                ps = psum.tile([128, 512], dt.float32, tag="ps", name="ps")
                for j in range(8):
                    nc.tensor.matmul(ps[:], oTs[j][:, qr], wo_sb[j][:, c],
                                     start=(j == 0), stop=(j == 7))
                ot = opool.tile([128, 512], dt.float32, tag="osb", name="osb")
                nc.vector.scalar_tensor_tensor(
                    ot[:], ps[:], qm_sb[:, qt:qt + 1], bo_rep[:, c],
                    op0=ALU.mult, op1=ALU.add)
                nc.sync.dma_start(out_d[qr, c], ot[:])


def bq_or(bias_sb, j):
    return bias_sb[:, j:j + 1]


def get_nc():
    if "nc" not in _NC_CACHE:
        _NC_CACHE["nc"] = _build_nc()
    return _NC_CACHE["nc"]


def make_in_maps(q, x, q_mask, k_mask, Wq, bq, Wk, bk, Wv, bv, Wo, bo):
    """Host-side shard/layout prep. Returns in_maps for cores 0..7."""
    wq_b = Wq.astype(BF16)
    wk_b = Wk.astype(BF16)
    wv_b = Wv.astype(BF16)
    wo_b = Wo.astype(BF16)
    bq_p = np.ascontiguousarray(bq.astype(np.float32).reshape(8, 128).T)
    bk_p = np.ascontiguousarray(bk.astype(np.float32).reshape(8, 128).T)
    bv_r = bv.astype(BF16).reshape(1, D)
    bo_r = bo.astype(BF16).reshape(1, D)
    in_maps = []
    for c in range(NCORES):
        b, qh = c // 2, c % 2
        qs = slice(QS * qh, QS * (qh + 1))
        kbias = np.where(k_mask[b] != 0, 0.0, NEG).astype(np.float32)
        in_maps.append({
            "qT": np.ascontiguousarray(q[b, qs, :].T).astype(BF16),
            "xT": np.ascontiguousarray(x[b].T).astype(BF16),
            "Wq": wq_b, "Wk": wk_b, "Wv": wv_b, "Wo": wo_b,
            "bqp": bq_p, "bkp": bk_p, "bvr": bv_r, "bor": bo_r,
            "kb": np.ascontiguousarray(kbias.reshape(8, 128).T),
            "qmc": np.ascontiguousarray(q_mask[b, qs].astype(np.float32).reshape(4, 128).T),
        })
    return in_maps


def kernel(q, x, q_mask, k_mask, Wq, bq, Wk, bk, Wv, bv, Wo, bo):
    from concourse import bass_utils

    q = np.asarray(q, np.float32)
    x = np.asarray(x, np.float32)
    q_mask = np.asarray(q_mask)
    k_mask = np.asarray(k_mask)

    nc = get_nc()
    in_maps = make_in_maps(q, x, q_mask, k_mask, Wq, bq, Wk, bk, Wv, bv, Wo, bo)
    res = bass_utils.run_bass_kernel_spmd(nc, in_maps, core_ids=list(range(NCORES)))

    out = np.empty((B, LQ, D), np.float32)
    for c in range(NCORES):
        b, qh = c // 2, c % 2
        out[b, QS * qh:QS * (qh + 1), :] = res.results[c]["out"]
    return out


# revision 22
# speedup vs baseline: 1.1233x; 1.0737x over previous
"""Multi-head attention (B=4, L=1024, D=1024, H=16) on 8 TRN2 NeuronCores.

Sharding: pure data-parallel over (batch, query-half) — core c handles batch
c//2, query rows [512*(c%2), 512*(c%2+1)). Each core computes Q/K/V
projections for its batch (K/V duplicated across the 2 cores of a batch),
full attention for its 512 queries, and the output projection for its slice.
No collectives; the host concatenates the 8 output slices.

Everything on-device is kept in transposed layouts so no transposes are ever
needed:
  Q^T[vd, q]  = Wq(lhsT) @ qT(rhs)         (+bq per-partition via activation)
  K^T[vd, k]  = Wk(lhsT) @ xT(rhs)         (+bk per-partition)
  V  [k, vd]  = xT(lhsT) @ Wv(rhs)         (+bv via K=1 ones-row matmul)
  S^T[k, q]   = K^T_h(lhsT, K=64) @ Q^T_h  per head
  expS        = exp(S^T/8 + kmask_bias)    (ScalarE, PSUM->SBUF bf16)
  O^T+denom   = V_aug(lhsT, M=65) @ expS   (V cols + ones col per head)
  scale       = q_mask * 1/denom, broadcast 16->64 partitions via selector MM
  out[q, d]   = O^T_scaled(lhsT) @ Wo      (+bo via K=1 ones-row matmul)
"""

import os

os.environ.setdefault("MYCRO_LOCAL_CACHE", "1")

import numpy as np
import ml_dtypes

BF16 = ml_dtypes.bfloat16

B, LQ, LK = 4, 1024, 1024
D = 1024  # QD = KD = VD
H, DH = 16, 64
QS = 512  # queries per core
NCORES = 8
NEG = -1e4  # additive key-mask bias (exp(-1e4) == 0 in f32)

_NC_CACHE = {}


def _build_nc(debug_taps=False):
    import concourse.bacc as bacc
    import concourse.mybir as mybir
    import concourse.tile as tile

    dt = mybir.dt
    AF = mybir.ActivationFunctionType

    nc = bacc.Bacc(
        "TRN2",
        debug=False,
        target_bir_lowering=False,
        num_devices=NCORES,
    )

    def din(name, shape, dtype):
        return nc.dram_tensor(name, shape, dtype, kind="ExternalInput").ap()

    qT_d = din("qT", [D, QS], dt.bfloat16)
    xT_d = din("xT", [D, LK], dt.bfloat16)
    wq_d = din("Wq", [D, D], dt.bfloat16)
    wk_d = din("Wk", [D, D], dt.bfloat16)
    wv_d = din("Wv", [D, D], dt.bfloat16)
    wo_d = din("Wo", [D, D], dt.bfloat16)
    bq_d = din("bqp", [128, 8], dt.float32)  # bq[t*128+p] at [p, t]
    bk_d = din("bkp", [128, 8], dt.float32)
    bv_d = din("bvr", [1, D], dt.bfloat16)
    bo_d = din("bor", [1, D], dt.bfloat16)
    kb_d = din("kb", [128, 8], dt.float32)  # key-mask bias, [p, t]
    qm_d = din("qmc", [128, 4], dt.float32)  # q_mask, [p, qt]
    out_d = nc.dram_tensor("out", [QS, D], dt.float32, kind="ExternalOutput").ap()

    taps = None
    if debug_taps:
        taps = {
            "dbg_qTp": nc.dram_tensor("dbg_qTp", [128, QS], dt.bfloat16, kind="ExternalOutput").ap(),
            "dbg_kT": nc.dram_tensor("dbg_kT", [128, LK], dt.bfloat16, kind="ExternalOutput").ap(),
            "dbg_v": nc.dram_tensor("dbg_v", [128, H * (DH + 1)], dt.bfloat16, kind="ExternalOutput").ap(),
            "dbg_oTs": nc.dram_tensor("dbg_oTs", [128, QS], dt.bfloat16, kind="ExternalOutput").ap(),
        }

    with tile.TileContext(nc) as tc:
        _body(tc, dt, AF, qT_d, xT_d, wq_d, wk_d, wv_d, wo_d, bq_d, bk_d,
              bv_d, bo_d, kb_d, qm_d, out_d, taps)

    nc.compile()
    return nc


def _body(tc, dt, AF, qT_d, xT_d, wq_d, wk_d, wv_d, wo_d, bq_d, bk_d,
          bv_d, bo_d, kb_d, qm_d, out_d, taps=None):
    from contextlib import ExitStack

    import concourse.mybir as mybir

    ALU = mybir.AluOpType
    nc = tc.nc
    with ExitStack() as ctx:
        const = ctx.enter_context(tc.tile_pool(name="const", bufs=1))
        espool = ctx.enter_context(tc.tile_pool(name="es", bufs=8))
        psum = ctx.enter_context(tc.tile_pool(name="psum", bufs=8, space="PSUM"))
        opool = ctx.enter_context(tc.tile_pool(name="osb", bufs=3))
        srpool = ctx.enter_context(tc.tile_pool(name="srp", bufs=2))

        def ctile(shape, dtype, tag):
            return const.tile(shape, dtype, tag=tag, name=tag)

        # ---- persistent SBUF tensors + input DMA ----
        # order matters: V/K projections start as soon as xT/wv/wk tiles land
        qT_sb = [ctile([128, QS], dt.bfloat16, f"qT{t}") for t in range(8)]
        wq_sb = [ctile([128, D], dt.bfloat16, f"wq{t}") for t in range(8)]
        xT_sb = [ctile([128, LK], dt.bfloat16, f"xT{t}") for t in range(8)]
        wk_sb = [ctile([128, D], dt.bfloat16, f"wk{t}") for t in range(8)]
        wv_sb = [ctile([128, D], dt.bfloat16, f"wv{t}") for t in range(8)]
        wo_sb = [ctile([128, D], dt.bfloat16, f"wo{t}") for t in range(8)]
        de = [nc.sync, nc.scalar, nc.gpsimd, nc.sync]
        for t in range(8):
            r = slice(128 * t, 128 * (t + 1))
            de[t % 4].dma_start(xT_sb[t][:], xT_d[r, :])
            de[(t + 1) % 4].dma_start(wv_sb[t][:], wv_d[r, :])
            de[(t + 2) % 4].dma_start(wk_sb[t][:], wk_d[r, :])
        for t in range(8):
            r = slice(128 * t, 128 * (t + 1))
            de[t % 4].dma_start(qT_sb[t][:], qT_d[r, :])
            de[(t + 1) % 4].dma_start(wq_sb[t][:], wq_d[r, :])

        bq_sb = ctile([128, 8], dt.float32, "bq")
        bk_sb = ctile([128, 8], dt.float32, "bk")
        bv_sb = ctile([1, D], dt.bfloat16, "bv")
        bo_sb = ctile([1, D], dt.bfloat16, "bo")
        kb_sb = ctile([128, 8], dt.float32, "kb")
        qm_sb = ctile([128, 4], dt.float32, "qm")  # q_mask, [p, qt]
        nc.sync.dma_start(bq_sb[:], bq_d[:, :])
        nc.sync.dma_start(bk_sb[:], bk_d[:, :])
        nc.sync.dma_start(bv_sb[:], bv_d[:, :])
        nc.sync.dma_start(bo_sb[:], bo_d[:, :])
        nc.sync.dma_start(kb_sb[:], kb_d[:, :])
        nc.sync.dma_start(qm_sb[:], qm_d[:, :])
        for t in range(8):
            r = slice(128 * t, 128 * (t + 1))
            de[(t + 3) % 4].dma_start(wo_sb[t][:], wo_d[r, :])

        ones1 = ctile([1, 128], dt.bfloat16, "ones1")
        nc.gpsimd.memset(ones1[:], 1.0)

        # bo broadcast to all partitions (final tiles add it with DVE)
        bo_rep = ctile([128, D], dt.float32, "bo_rep")
        for n in range(2):
            c = slice(512 * n, 512 * (n + 1))
            ps = psum.tile([128, 512], dt.float32, tag="ps", name="ps")
            nc.tensor.matmul(ps[:], ones1[:], bo_sb[:, c], start=True, stop=True)
            nc.vector.tensor_copy(bo_rep[:, c], ps[:])

        # ---- V projection into V_aug layout: per k-tile [128, 16*(64+1)] ----
        # head h occupies cols [65h, 65h+64) = V[:, 64h:64h+64]; col 65h+64 = 1.
        v_sb = [ctile([128, H * (DH + 1)], dt.bfloat16, f"v{t}") for t in range(8)]
        for t in range(8):
            ones_cols = v_sb[t][:].rearrange("p (h c) -> p h c", c=DH + 1)[:, :, DH:DH + 1]
            nc.gpsimd.memset(ones_cols, 1.0)
        for t in range(8):
            for n in range(2):
                c = slice(512 * n, 512 * (n + 1))
                ps = psum.tile([128, 512], dt.float32, tag="ps", name="ps")
                for kd in range(8):
                    nc.tensor.matmul(
                        ps[:], xT_sb[kd][:, 128 * t:128 * (t + 1)], wv_sb[kd][:, c],
                        start=(kd == 0), stop=False)
                nc.tensor.matmul(ps[:], ones1[:], bv_sb[:, c],
                                 start=False, stop=True)
                for i in range(8):
                    h = 8 * n + i
                    nc.vector.tensor_copy(
                        v_sb[t][:, 65 * h:65 * h + 64], ps[:, 64 * i:64 * (i + 1)])

        # ---- per head-pair: K^T/Q^T projection for its vd-tile, then attention
        kT_sb = [ctile([128, LK], dt.bfloat16, f"kT{j}") for j in range(8)]
        qTp = [ctile([128, QS], dt.bfloat16, f"qTp{j}") for j in range(8)]
        oTs = [ctile([128, QS], dt.bfloat16, f"oTs{j}") for j in range(8)]
        dscr = ctile([1, 2 * QS], dt.float32, "dscr")
        sca = ctile([1, 2 * QS], dt.float32, "sca")
        rscr = ctile([1, 2 * QS], dt.float32, "rscr")
        ones64f = ctile([1, 64], dt.float32, "ones64f")
        nc.gpsimd.memset(ones64f[:], 1.0)

        for j in range(8):
            hA, hB = 2 * j, 2 * j + 1
            # K^T tile j (+bk via DVE per-partition add)
            for n in range(2):
                c = slice(512 * n, 512 * (n + 1))
                ps = psum.tile([128, 512], dt.float32, tag="ps", name="ps")
                for kt in range(8):
                    nc.tensor.matmul(
                        ps[:], wk_sb[kt][:, 128 * j:128 * (j + 1)], xT_sb[kt][:, c],
                        start=(kt == 0), stop=(kt == 7))
                nc.vector.tensor_scalar_add(kT_sb[j][:, c], ps[:], bq_or(bk_sb, j))
            # Q^T tile j (+bq)
            ps = psum.tile([128, QS], dt.float32, tag="ps", name="ps")
            for kt in range(8):
                nc.tensor.matmul(
                    ps[:], wq_sb[kt][:, 128 * j:128 * (j + 1)], qT_sb[kt][:],
                    start=(kt == 0), stop=(kt == 7))
            nc.vector.tensor_scalar_add(qTp[j][:], ps[:], bq_or(bq_sb, j))

            # attention for heads (2j, 2j+1); S/exp run one k-tile ahead
            # of the O accumulation so the PE never waits on the exp
            oA = psum.tile([128, QS], dt.float32, tag="ps", name="ps")
            oB = psum.tile([128, QS], dt.float32, tag="ps", name="ps")
            es_tiles = {}

            def s_stage(kt, j=j):
                kc = slice(128 * kt, 128 * (kt + 1))
                sA = psum.tile([128, QS], dt.float32, tag="ps", name="ps")
                nc.tensor.matmul(sA[:], kT_sb[j][0:64, kc], qTp[j][0:64, :],
                                 start=True, stop=True)
                eA = espool.tile([128, QS], dt.bfloat16, tag="es", name="es")
                nc.scalar.activation(eA[:], sA[:], AF.Exp,
                                     bias=kb_sb[:, kt:kt + 1], scale=0.125)
                sB = psum.tile([128, QS], dt.float32, tag="ps", name="ps")
                nc.tensor.matmul(sB[:], kT_sb[j][64:128, kc], qTp[j][64:128, :],
                                 start=True, stop=True)
                eB = espool.tile([128, QS], dt.bfloat16, tag="es", name="es")
                nc.scalar.activation(eB[:], sB[:], AF.Exp,
                                     bias=kb_sb[:, kt:kt + 1], scale=0.125)
                es_tiles[kt] = (eA, eB)

            def o_stage(kt, j=j, hA=hA, hB=hB, oA=oA, oB=oB):
                eA, eB = es_tiles.pop(kt)
                nc.tensor.matmul(oA[0:65, :], v_sb[kt][:, 65 * hA:65 * hA + 65],
                                 eA[:], start=(kt == 0), stop=(kt == 7))
                nc.tensor.matmul(oB[0:65, :], v_sb[kt][:, 65 * hB:65 * hB + 65],
                                 eB[:], start=(kt == 0), stop=(kt == 7))

            s_stage(0)
            for kt in range(1, 8):
                s_stage(kt)
                o_stage(kt - 1)
            o_stage(7)

            # row 64 = denominator; 1/denom via ScalarE LUT, broadcast to the
            # 64 partitions of each head with a K=1 ones matmul, then apply
            nc.vector.tensor_copy(dscr[0:1, 0:QS], oA[64:65, :])
            nc.vector.tensor_copy(dscr[0:1, QS:2 * QS], oB[64:65, :])
            nc.vector.reciprocal_approx_accurate(out=sca[:], in_=dscr[:],
                                                 scratch=rscr[:])
            sr = psum.tile([128, QS], dt.float32, tag="ps", name="ps")
            nc.tensor.matmul(sr[0:64, :], ones64f[:], sca[:, 0:QS],
                             start=True, stop=True)
            nc.tensor.matmul(sr[64:128, :], ones64f[:], sca[:, QS:2 * QS],
                             start=True, stop=True, tile_position=(0, 64))
            srs = srpool.tile([128, QS], dt.float32, tag="srs", name="srs")
            nc.vector.tensor_copy(srs[:], sr[:])
            nc.vector.tensor_mul(oTs[j][0:64, :], oA[0:64, :], srs[0:64, :])
            nc.vector.tensor_mul(oTs[j][64:128, :], oB[0:64, :], srs[64:128, :])

        if taps is not None:
            nc.sync.dma_start(taps["dbg_qTp"][:, :], qTp[0][:])
            nc.sync.dma_start(taps["dbg_kT"][:, :], kT_sb[0][:])
            nc.sync.dma_start(taps["dbg_v"][:, :], v_sb[0][:])
            nc.sync.dma_start(taps["dbg_oTs"][:, :], oTs[0][:])

        # ---- output projection: out[q, d] = (O^T.T @ Wo) * q_mask + bo ----
        for qt in range(4):
            qr = slice(128 * qt, 128 * (qt + 1))
            for n in range(2):
                c = slice(512 * n, 512 * (n + 1))
                ps = psum.tile([128, 512], dt.float32, tag="ps", name="ps")
                for j in range(8):
                    nc.tensor.matmul(ps[:], oTs[j][:, qr], wo_sb[j][:, c],
                                     start=(j == 0), stop=(j == 7))
                ot = opool.tile([128, 512], dt.float32, tag="osb", name="osb")
                nc.vector.scalar_tensor_tensor(
                    ot[:], ps[:], qm_sb[:, qt:qt + 1], bo_rep[:, c],
                    op0=ALU.mult, op1=ALU.add)
                nc.sync.dma_start(out_d[qr, c], ot[:])


def bq_or(bias_sb, j):
    return bias_sb[:, j:j + 1]


def get_nc():
    if "nc" not in _NC_CACHE:
        _NC_CACHE["nc"] = _build_nc()
    return _NC_CACHE["nc"]


def make_in_maps(q, x, q_mask, k_mask, Wq, bq, Wk, bk, Wv, bv, Wo, bo):
    """Host-side shard/layout prep. Returns in_maps for cores 0..7."""
    wq_b = Wq.astype(BF16)
    wk_b = Wk.astype(BF16)
    wv_b = Wv.astype(BF16)
    wo_b = Wo.astype(BF16)
    bq_p = np.ascontiguousarray(bq.astype(np.float32).reshape(8, 128).T)
    bk_p = np.ascontiguousarray(bk.astype(np.float32).reshape(8, 128).T)
    bv_r = bv.astype(BF16).reshape(1, D)
    bo_r = bo.astype(BF16).reshape(1, D)
    in_maps = []
    for c in range(NCORES):
        b, qh = c // 2, c % 2
        qs = slice(QS * qh, QS * (qh + 1))
        kbias = np.where(k_mask[b] != 0, 0.0, NEG).astype(np.float32)
        in_maps.append({
            "qT": np.ascontiguousarray(q[b, qs, :].T).astype(BF16),
            "xT": np.ascontiguousarray(x[b].T).astype(BF16),
            "Wq": wq_b, "Wk": wk_b, "Wv": wv_b, "Wo": wo_b,
            "bqp": bq_p, "bkp": bk_p, "bvr": bv_r, "bor": bo_r,
            "kb": np.ascontiguousarray(kbias.reshape(8, 128).T),
            "qmc": np.ascontiguousarray(q_mask[b, qs].astype(np.float32).reshape(4, 128).T),
        })
    return in_maps


def kernel(q, x, q_mask, k_mask, Wq, bq, Wk, bk, Wv, bv, Wo, bo):
    from concourse import bass_utils

    q = np.asarray(q, np.float32)
    x = np.asarray(x, np.float32)
    q_mask = np.asarray(q_mask)
    k_mask = np.asarray(k_mask)

    nc = get_nc()
    in_maps = make_in_maps(q, x, q_mask, k_mask, Wq, bq, Wk, bk, Wv, bv, Wo, bo)
    res = bass_utils.run_bass_kernel_spmd(nc, in_maps, core_ids=list(range(NCORES)))

    out = np.empty((B, LQ, D), np.float32)
    for c in range(NCORES):
        b, qh = c // 2, c % 2
        out[b, QS * qh:QS * (qh + 1), :] = res.results[c]["out"]
    return out


# revision 23
# speedup vs baseline: 1.1790x; 1.0496x over previous
"""Multi-head attention (B=4, L=1024, D=1024, H=16) on 8 TRN2 NeuronCores.

Sharding: pure data-parallel over (batch, query-half) — core c handles batch
c//2, query rows [512*(c%2), 512*(c%2+1)). Each core computes Q/K/V
projections for its batch (K/V duplicated across the 2 cores of a batch),
full attention for its 512 queries, and the output projection for its slice.
No collectives; the host concatenates the 8 output slices.

Everything on-device is kept in transposed layouts so no transposes are ever
needed:
  Q^T[vd, q]  = Wq(lhsT) @ qT(rhs)         (+bq per-partition via activation)
  K^T[vd, k]  = Wk(lhsT) @ xT(rhs)         (+bk per-partition)
  V  [k, vd]  = xT(lhsT) @ Wv(rhs)         (+bv via K=1 ones-row matmul)
  S^T[k, q]   = K^T_h(lhsT, K=64) @ Q^T_h  per head
  expS        = exp(S^T/8 + kmask_bias)    (ScalarE, PSUM->SBUF bf16)
  O^T+denom   = V_aug(lhsT, M=65) @ expS   (V cols + ones col per head)
  scale       = q_mask * 1/denom, broadcast 16->64 partitions via selector MM
  out[q, d]   = O^T_scaled(lhsT) @ Wo      (+bo via K=1 ones-row matmul)
"""

import os

os.environ.setdefault("MYCRO_LOCAL_CACHE", "1")

import numpy as np
import ml_dtypes

BF16 = ml_dtypes.bfloat16

B, LQ, LK = 4, 1024, 1024
D = 1024  # QD = KD = VD
H, DH = 16, 64
QS = 512  # queries per core
NCORES = 8
NEG = -1e4  # additive key-mask bias (exp(-1e4) == 0 in f32)

_NC_CACHE = {}


def _build_nc(debug_taps=False):
    import concourse.bacc as bacc
    import concourse.mybir as mybir
    import concourse.tile as tile

    dt = mybir.dt
    AF = mybir.ActivationFunctionType

    nc = bacc.Bacc(
        "TRN2",
        debug=False,
        target_bir_lowering=False,
        num_devices=NCORES,
    )

    def din(name, shape, dtype):
        return nc.dram_tensor(name, shape, dtype, kind="ExternalInput").ap()

    qT_d = din("qT", [D, QS], dt.bfloat16)
    xT_d = din("xT", [D, LK], dt.bfloat16)
    wq_d = din("Wq", [D, D], dt.bfloat16)
    wk_d = din("Wk", [D, D], dt.bfloat16)
    wv_d = din("Wv", [D, D], dt.bfloat16)
    wo_d = din("Wo", [D, D], dt.bfloat16)
    bq_d = din("bqp", [128, 8], dt.float32)  # bq[t*128+p] at [p, t]
    bk_d = din("bkp", [128, 8], dt.float32)
    bv_d = din("bvr", [1, D], dt.bfloat16)
    bo_d = din("bor", [1, D], dt.bfloat16)
    kb_d = din("kb", [128, 8], dt.float32)  # key-mask bias, [p, t]
    qm_d = din("qmc", [128, 4], dt.float32)  # q_mask, [p, qt]
    out_d = nc.dram_tensor("out", [QS, D], dt.float32, kind="ExternalOutput").ap()

    taps = None
    if debug_taps:
        taps = {
            "dbg_qTp": nc.dram_tensor("dbg_qTp", [128, QS], dt.bfloat16, kind="ExternalOutput").ap(),
            "dbg_kT": nc.dram_tensor("dbg_kT", [128, LK], dt.bfloat16, kind="ExternalOutput").ap(),
            "dbg_v": nc.dram_tensor("dbg_v", [128, H * (DH + 1)], dt.bfloat16, kind="ExternalOutput").ap(),
            "dbg_oTs": nc.dram_tensor("dbg_oTs", [128, QS], dt.bfloat16, kind="ExternalOutput").ap(),
        }

    with tile.TileContext(nc) as tc:
        _body(tc, dt, AF, qT_d, xT_d, wq_d, wk_d, wv_d, wo_d, bq_d, bk_d,
              bv_d, bo_d, kb_d, qm_d, out_d, taps)

    nc.compile()
    return nc


def _body(tc, dt, AF, qT_d, xT_d, wq_d, wk_d, wv_d, wo_d, bq_d, bk_d,
          bv_d, bo_d, kb_d, qm_d, out_d, taps=None):
    from contextlib import ExitStack

    import concourse.mybir as mybir

    ALU = mybir.AluOpType
    nc = tc.nc
    with ExitStack() as ctx:
        const = ctx.enter_context(tc.tile_pool(name="const", bufs=1))
        espool = ctx.enter_context(tc.tile_pool(name="es", bufs=8))
        psum = ctx.enter_context(tc.tile_pool(name="psum", bufs=8, space="PSUM"))
        opool = ctx.enter_context(tc.tile_pool(name="osb", bufs=3))
        srpool = ctx.enter_context(tc.tile_pool(name="srp", bufs=2))

        def ctile(shape, dtype, tag):
            return const.tile(shape, dtype, tag=tag, name=tag)

        # ---- persistent SBUF tensors + input DMA ----
        # order matters: V/K projections start as soon as xT/wv/wk tiles land
        qT_sb = [ctile([128, QS], dt.bfloat16, f"qT{t}") for t in range(8)]
        wq_sb = [ctile([128, D], dt.bfloat16, f"wq{t}") for t in range(8)]
        xT_sb = [ctile([128, LK], dt.bfloat16, f"xT{t}") for t in range(8)]
        wk_sb = [ctile([128, D], dt.bfloat16, f"wk{t}") for t in range(8)]
        wv_sb = [ctile([128, D], dt.bfloat16, f"wv{t}") for t in range(8)]
        wo_sb = [ctile([128, D], dt.bfloat16, f"wo{t}") for t in range(8)]
        de = [nc.sync, nc.scalar, nc.gpsimd]
        for t in range(8):
            r = slice(128 * t, 128 * (t + 1))
            de[t % 3].dma_start(xT_sb[t][:], xT_d[r, :])
            de[(t + 1) % 3].dma_start(wv_sb[t][:], wv_d[r, :])
            de[(t + 2) % 3].dma_start(wk_sb[t][:], wk_d[r, :])
        for t in range(8):
            r = slice(128 * t, 128 * (t + 1))
            de[t % 3].dma_start(qT_sb[t][:], qT_d[r, :])
            de[(t + 1) % 3].dma_start(wq_sb[t][:], wq_d[r, :])

        bq_sb = ctile([128, 8], dt.float32, "bq")
        bk_sb = ctile([128, 8], dt.float32, "bk")
        bv_sb = ctile([1, D], dt.bfloat16, "bv")
        bo_sb = ctile([1, D], dt.bfloat16, "bo")
        kb_sb = ctile([128, 8], dt.float32, "kb")
        qm_sb = ctile([128, 4], dt.float32, "qm")  # q_mask, [p, qt]
        nc.sync.dma_start(bq_sb[:], bq_d[:, :])
        nc.sync.dma_start(bk_sb[:], bk_d[:, :])
        nc.sync.dma_start(bv_sb[:], bv_d[:, :])
        nc.sync.dma_start(bo_sb[:], bo_d[:, :])
        nc.sync.dma_start(kb_sb[:], kb_d[:, :])
        nc.sync.dma_start(qm_sb[:], qm_d[:, :])
        for t in range(8):
            r = slice(128 * t, 128 * (t + 1))
            de[(t + 2) % 3].dma_start(wo_sb[t][:], wo_d[r, :])

        ones1 = ctile([1, 128], dt.bfloat16, "ones1")
        nc.gpsimd.memset(ones1[:], 1.0)

        # bo broadcast to all partitions (final tiles add it with DVE)
        bo_rep = ctile([128, D], dt.float32, "bo_rep")
        for n in range(2):
            c = slice(512 * n, 512 * (n + 1))
            ps = psum.tile([128, 512], dt.float32, tag="ps", name="ps")
            nc.tensor.matmul(ps[:], ones1[:], bo_sb[:, c], start=True, stop=True)
            nc.vector.tensor_copy(bo_rep[:, c], ps[:])

        # ---- V projection into V_aug layout: per k-tile [128, 16*(64+1)] ----
        # head h occupies cols [65h, 65h+64) = V[:, 64h:64h+64]; col 65h+64 = 1.
        v_sb = [ctile([128, H * (DH + 1)], dt.bfloat16, f"v{t}") for t in range(8)]
        for t in range(8):
            ones_cols = v_sb[t][:].rearrange("p (h c) -> p h c", c=DH + 1)[:, :, DH:DH + 1]
            nc.gpsimd.memset(ones_cols, 1.0)
        for t in range(8):
            for n in range(2):
                c = slice(512 * n, 512 * (n + 1))
                ps = psum.tile([128, 512], dt.float32, tag="ps", name="ps")
                for kd in range(8):
                    nc.tensor.matmul(
                        ps[:], xT_sb[kd][:, 128 * t:128 * (t + 1)], wv_sb[kd][:, c],
                        start=(kd == 0), stop=False)
                nc.tensor.matmul(ps[:], ones1[:], bv_sb[:, c],
                                 start=False, stop=True)
                for i in range(8):
                    h = 8 * n + i
                    nc.vector.tensor_copy(
                        v_sb[t][:, 65 * h:65 * h + 64], ps[:, 64 * i:64 * (i + 1)])

        # ---- per head-pair: K^T/Q^T projection for its vd-tile, then attention
        kT_sb = [ctile([128, LK], dt.bfloat16, f"kT{j}") for j in range(8)]
        qTp = [ctile([128, QS], dt.bfloat16, f"qTp{j}") for j in range(8)]
        oTs = [ctile([128, QS], dt.bfloat16, f"oTs{j}") for j in range(8)]
        dscr = ctile([1, 2 * QS], dt.float32, "dscr")
        sca = ctile([1, 2 * QS], dt.float32, "sca")
        scb = ctile([1, 2 * QS], dt.bfloat16, "scb")
        rscr = ctile([1, 2 * QS], dt.float32, "rscr")
        ones64 = ctile([1, 64], dt.bfloat16, "ones64")
        nc.gpsimd.memset(ones64[:], 1.0)

        for j in range(8):
            hA, hB = 2 * j, 2 * j + 1
            # K^T tile j (+bk via DVE per-partition add)
            for n in range(2):
                c = slice(512 * n, 512 * (n + 1))
                ps = psum.tile([128, 512], dt.float32, tag="ps", name="ps")
                for kt in range(8):
                    nc.tensor.matmul(
                        ps[:], wk_sb[kt][:, 128 * j:128 * (j + 1)], xT_sb[kt][:, c],
                        start=(kt == 0), stop=(kt == 7))
                nc.vector.tensor_scalar_add(kT_sb[j][:, c], ps[:], bq_or(bk_sb, j))
            # Q^T tile j (+bq)
            ps = psum.tile([128, QS], dt.float32, tag="ps", name="ps")
            for kt in range(8):
                nc.tensor.matmul(
                    ps[:], wq_sb[kt][:, 128 * j:128 * (j + 1)], qT_sb[kt][:],
                    start=(kt == 0), stop=(kt == 7))
            nc.vector.tensor_scalar_add(qTp[j][:], ps[:], bq_or(bq_sb, j))

            # attention for heads (2j, 2j+1); S/exp run one k-tile ahead
            # of the O accumulation so the PE never waits on the exp
            oA = psum.tile([128, QS], dt.float32, tag="ps", name="ps")
            oB = psum.tile([128, QS], dt.float32, tag="ps", name="ps")
            es_tiles = {}

            def s_stage(kt, j=j):
                kc = slice(128 * kt, 128 * (kt + 1))
                sA = psum.tile([128, QS], dt.float32, tag="ps", name="ps")
                nc.tensor.matmul(sA[:], kT_sb[j][0:64, kc], qTp[j][0:64, :],
                                 start=True, stop=True)
                eA = espool.tile([128, QS], dt.bfloat16, tag="es", name="es")
                nc.scalar.activation(eA[:], sA[:], AF.Exp,
                                     bias=kb_sb[:, kt:kt + 1], scale=0.125)
                sB = psum.tile([128, QS], dt.float32, tag="ps", name="ps")
                nc.tensor.matmul(sB[:], kT_sb[j][64:128, kc], qTp[j][64:128, :],
                                 start=True, stop=True)
                eB = espool.tile([128, QS], dt.bfloat16, tag="es", name="es")
                nc.scalar.activation(eB[:], sB[:], AF.Exp,
                                     bias=kb_sb[:, kt:kt + 1], scale=0.125)
                es_tiles[kt] = (eA, eB)

            def o_stage(kt, j=j, hA=hA, hB=hB, oA=oA, oB=oB):
                eA, eB = es_tiles.pop(kt)
                nc.tensor.matmul(oA[0:65, :], v_sb[kt][:, 65 * hA:65 * hA + 65],
                                 eA[:], start=(kt == 0), stop=(kt == 7))
                nc.tensor.matmul(oB[0:65, :], v_sb[kt][:, 65 * hB:65 * hB + 65],
                                 eB[:], start=(kt == 0), stop=(kt == 7))

            s_stage(0)
            for kt in range(1, 8):
                s_stage(kt)
                o_stage(kt - 1)
            o_stage(7)

            # row 64 = denominator; 1/denom via ScalarE LUT, broadcast to the
            # 64 partitions of each head with a K=1 ones matmul, then apply
            nc.vector.tensor_copy(dscr[0:1, 0:QS], oA[64:65, :])
            nc.vector.tensor_copy(dscr[0:1, QS:2 * QS], oB[64:65, :])
            nc.vector.reciprocal_approx_accurate(out=sca[:], in_=dscr[:],
                                                 scratch=rscr[:])
            nc.vector.tensor_copy(scb[:], sca[:])
            sr = psum.tile([128, QS], dt.float32, tag="ps", name="ps")
            nc.tensor.matmul(sr[0:64, :], ones64[:], scb[:, 0:QS],
                             start=True, stop=True)
            nc.tensor.matmul(sr[64:128, :], ones64[:], scb[:, QS:2 * QS],
                             start=True, stop=True, tile_position=(0, 64))
            srs = srpool.tile([128, QS], dt.float32, tag="srs", name="srs")
            nc.vector.tensor_copy(srs[:], sr[:])
            nc.vector.tensor_mul(oTs[j][0:64, :], oA[0:64, :], srs[0:64, :])
            nc.vector.tensor_mul(oTs[j][64:128, :], oB[0:64, :], srs[64:128, :])

        if taps is not None:
            nc.sync.dma_start(taps["dbg_qTp"][:, :], qTp[0][:])
            nc.sync.dma_start(taps["dbg_kT"][:, :], kT_sb[0][:])
            nc.sync.dma_start(taps["dbg_v"][:, :], v_sb[0][:])
            nc.sync.dma_start(taps["dbg_oTs"][:, :], oTs[0][:])

        # ---- output projection: out[q, d] = (O^T.T @ Wo) * q_mask + bo ----
        for qt in range(4):
            qr = slice(128 * qt, 128 * (qt + 1))
            for n in range(2):
                c = slice(512 * n, 512 * (n + 1))
                ps = psum.tile([128, 512], dt.float32, tag="ps", name="ps")
                for j in range(8):
                    nc.tensor.matmul(ps[:], oTs[j][:, qr], wo_sb[j][:, c],
                                     start=(j == 0), stop=(j == 7))
                ot = opool.tile([128, 512], dt.float32, tag="osb", name="osb")
                nc.vector.scalar_tensor_tensor(
                    ot[:], ps[:], qm_sb[:, qt:qt + 1], bo_rep[:, c],
                    op0=ALU.mult, op1=ALU.add)
                nc.sync.dma_start(out_d[qr, c], ot[:])


def bq_or(bias_sb, j):
    return bias_sb[:, j:j + 1]


def get_nc():
    if "nc" not in _NC_CACHE:
        _NC_CACHE["nc"] = _build_nc()
    return _NC_CACHE["nc"]


def make_in_maps(q, x, q_mask, k_mask, Wq, bq, Wk, bk, Wv, bv, Wo, bo):
    """Host-side shard/layout prep. Returns in_maps for cores 0..7."""
    wq_b = Wq.astype(BF16)
    wk_b = Wk.astype(BF16)
    wv_b = Wv.astype(BF16)
    wo_b = Wo.astype(BF16)
    bq_p = np.ascontiguousarray(bq.astype(np.float32).reshape(8, 128).T)
    bk_p = np.ascontiguousarray(bk.astype(np.float32).reshape(8, 128).T)
    bv_r = bv.astype(BF16).reshape(1, D)
    bo_r = bo.astype(BF16).reshape(1, D)
    in_maps = []
    for c in range(NCORES):
        b, qh = c // 2, c % 2
        qs = slice(QS * qh, QS * (qh + 1))
        kbias = np.where(k_mask[b] != 0, 0.0, NEG).astype(np.float32)
        in_maps.append({
            "qT": np.ascontiguousarray(q[b, qs, :].T).astype(BF16),
            "xT": np.ascontiguousarray(x[b].T).astype(BF16),
            "Wq": wq_b, "Wk": wk_b, "Wv": wv_b, "Wo": wo_b,
            "bqp": bq_p, "bkp": bk_p, "bvr": bv_r, "bor": bo_r,
            "kb": np.ascontiguousarray(kbias.reshape(8, 128).T),
            "qmc": np.ascontiguousarray(q_mask[b, qs].astype(np.float32).reshape(4, 128).T),
        })
    return in_maps


def kernel(q, x, q_mask, k_mask, Wq, bq, Wk, bk, Wv, bv, Wo, bo):
    from concourse import bass_utils

    q = np.asarray(q, np.float32)
    x = np.asarray(x, np.float32)
    q_mask = np.asarray(q_mask)
    k_mask = np.asarray(k_mask)

    nc = get_nc()
    in_maps = make_in_maps(q, x, q_mask, k_mask, Wq, bq, Wk, bk, Wv, bv, Wo, bo)
    res = bass_utils.run_bass_kernel_spmd(nc, in_maps, core_ids=list(range(NCORES)))

    out = np.empty((B, LQ, D), np.float32)
    for c in range(NCORES):
        b, qh = c // 2, c % 2
        out[b, QS * qh:QS * (qh + 1), :] = res.results[c]["out"]
    return out
